# revision 2
# baseline (speedup 1.0000x reference)
"""MoE branch-routing kernel for Trainium2 (8 NeuronCores).

Strategy: expert-parallel with host-side routing. The batch is grouped by
`command` (4 experts); each expert's rows are split across 2 of the 8 cores.
Every core runs the same static SPMD program: a dense 3-layer MLP
(relu(xW1+b1) -> relu(hW2+b2) -> tanh(hW3+b3)) over C rows with ITS OWN
expert's weights delivered via its per-core input map. Activations stay in
[feature, row] layout on-chip so no transposes are needed; the host
transposes x in / y out and scatters rows back to their original positions.

Matmuls run as float32r (full 1 cycle/row PE rate for free dim >= 256,
~1e-4 relative precision vs fp32).
"""

import numpy as np

B, D_IN, H, D_OUT, E = 16384, 512, 1024, 512, 4
N_CORES = 8
CORES_PER_EXPERT = N_CORES // E
KI, KH, MO = D_IN // 128, H // 128, D_OUT // 128  # 4, 8, 4

_CACHE = {}


def _make_blocks(C):
    """Split C rows into matmul moving-dim blocks <=512, preferring >=256
    (float32r runs at full PE rate only when the moving dim is >=256)."""
    blocks, r = [], C
    while r > 0:
        if r >= 768 or r <= 512:
            nb = min(512, r)
        else:  # 512 < r < 768: split evenly so both pieces are >= 256
            nb = (r // 2 + 31) // 32 * 32
        blocks.append(nb)
        r -= nb
    return blocks


def _build_program(C):
    import concourse.tile as tile
    from concourse import bacc, mybir

    f32, f32r = mybir.dt.float32, mybir.dt.float32r
    AFT = mybir.ActivationFunctionType

    nc = bacc.Bacc("TRN2", target_bir_lowering=False, debug=False,
                   num_devices=N_CORES)
    xT_d = nc.dram_tensor("xT", [D_IN, C], f32, kind="ExternalInput").ap()
    w1_d = nc.dram_tensor("w1", [D_IN, H], f32, kind="ExternalInput").ap()
    w2_d = nc.dram_tensor("w2", [H, H], f32, kind="ExternalInput").ap()
    w3_d = nc.dram_tensor("w3", [H, D_OUT], f32, kind="ExternalInput").ap()
    b1_d = nc.dram_tensor("b1t", [128, KH], f32, kind="ExternalInput").ap()
    b2_d = nc.dram_tensor("b2t", [128, KH], f32, kind="ExternalInput").ap()
    b3_d = nc.dram_tensor("b3t", [128, MO], f32, kind="ExternalInput").ap()
    yT_d = nc.dram_tensor("yT", [D_OUT, C], f32, kind="ExternalOutput").ap()

    x_re = xT_d.rearrange("(k p) c -> p k c", p=128).bitcast(f32r)
    w1_re = w1_d.rearrange("(k p) h -> p k h", p=128).bitcast(f32r)
    w2_re = w2_d.rearrange("(k p) h -> p k h", p=128).bitcast(f32r)
    w3_re = w3_d.rearrange("(k p) h -> p k h", p=128).bitcast(f32r)
    y_re = yT_d.rearrange("(m p) c -> p m c", p=128)

    blocks = _make_blocks(C)

    with tile.TileContext(nc) as tc:
        with tc.tile_pool(name="wpool", bufs=1) as wpool, \
             tc.tile_pool(name="xpool", bufs=3) as xpool, \
             tc.tile_pool(name="hpool", bufs=2) as hpool, \
             tc.tile_pool(name="ypool", bufs=2) as ypool, \
             tc.tile_pool(name="psum", bufs=8, space="PSUM") as psum:

            # Per-k-chunk weight loads so the first L1 matmuls only wait on
            # their own chunk, and W2/W3 stream in under L1 compute.
            w1sb = wpool.tile([128, KI, H], f32r)
            for k in range(KI):
                nc.sync.dma_start(w1sb[:, k, :], w1_re[:, k, :])
            b1sb = wpool.tile([128, KH], f32)
            nc.sync.dma_start(b1sb[:], b1_d[:])
            b2sb = wpool.tile([128, KH], f32)
            nc.sync.dma_start(b2sb[:], b2_d[:])
            b3sb = wpool.tile([128, MO], f32)
            nc.sync.dma_start(b3sb[:], b3_d[:])
            w2sb = wpool.tile([128, KH, H], f32r)
            for k in range(KH):
                nc.sync.dma_start(w2sb[:, k, :], w2_re[:, k, :])
            w3sb = wpool.tile([128, KH, D_OUT], f32r)
            for k in range(KH):
                nc.sync.dma_start(w3sb[:, k, :], w3_re[:, k, :])

            n0 = 0
            for nb in blocks:
                xts = []
                for k in range(KI):
                    xt = xpool.tile([128, 512], f32r, name=f"x{k}")
                    nc.sync.dma_start(xt[:, :nb], x_re[:, k, n0:n0 + nb])
                    xts.append(xt)

                # L1: h1 = relu(x @ W1 + b1), laid out [H, rows]
                h1 = []
                for m in range(KH):
                    pt = psum.tile([128, 512], f32, name="ps")
                    for k in range(KI):
                        nc.tensor.matmul(
                            pt[:, :nb],
                            lhsT=w1sb[:, k, m * 128:(m + 1) * 128],
                            rhs=xts[k][:, :nb],
                            start=(k == 0), stop=(k == KI - 1))
                    ht = hpool.tile([128, 512], f32r, name=f"h1_{m}")
                    nc.scalar.activation(ht[:, :nb], pt[:, :nb], AFT.Relu,
                                         bias=b1sb[:, m:m + 1])
                    h1.append(ht)

                # L2: h2 = relu(h1 @ W2 + b2)
                h2 = []
                for m in range(KH):
                    pt = psum.tile([128, 512], f32, name="ps")
                    for k in range(KH):
                        nc.tensor.matmul(
                            pt[:, :nb],
                            lhsT=w2sb[:, k, m * 128:(m + 1) * 128],
                            rhs=h1[k][:, :nb],
                            start=(k == 0), stop=(k == KH - 1))
                    ht = hpool.tile([128, 512], f32r, name=f"h2_{m}")
                    nc.scalar.activation(ht[:, :nb], pt[:, :nb], AFT.Relu,
                                         bias=b2sb[:, m:m + 1])
                    h2.append(ht)

                # L3: y = tanh(h2 @ W3 + b3), DMA out per m-chunk
                for m in range(MO):
                    pt = psum.tile([128, 512], f32, name="ps")
                    for k in range(KH):
                        nc.tensor.matmul(
                            pt[:, :nb],
                            lhsT=w3sb[:, k, m * 128:(m + 1) * 128],
                            rhs=h2[k][:, :nb],
                            start=(k == 0), stop=(k == KH - 1))
                    yt = ypool.tile([128, 512], f32, name=f"y{m}")
                    nc.scalar.activation(yt[:, :nb], pt[:, :nb], AFT.Tanh,
                                         bias=b3sb[:, m:m + 1])
                    nc.sync.dma_start(y_re[:, m, n0:n0 + nb], yt[:, :nb])
                n0 += nb

    nc.compile()
    return nc


def _prepare(x, command, W1, b1, W2, b2, W3, b3):
    """Route rows to cores and build the per-core input maps.

    Returns (nc, in_maps, core_rows, nrows)."""
    x = np.ascontiguousarray(np.asarray(x, dtype=np.float32))
    command = np.asarray(command).astype(np.int64)
    W1 = np.asarray(W1, dtype=np.float32)
    b1 = np.asarray(b1, dtype=np.float32)
    W2 = np.asarray(W2, dtype=np.float32)
    b2 = np.asarray(b2, dtype=np.float32)
    W3 = np.asarray(W3, dtype=np.float32)
    b3 = np.asarray(b3, dtype=np.float32)

    nrows = x.shape[0]
    order = np.argsort(command, kind="stable")
    counts = np.bincount(command, minlength=E)
    starts = np.concatenate([[0], np.cumsum(counts)])

    # Static per-core row capacity, shared by all cores (one SPMD program).
    C = int(-(-int(counts.max()) // CORES_PER_EXPERT))
    C = max(256, -(-C // 64) * 64)

    if C not in _CACHE:
        _CACHE[C] = _build_program(C)
    nc = _CACHE[C]

    xT = x.T  # [D_IN, B] view
    in_maps = []
    core_rows = []
    for e in range(E):
        rows_e = order[starts[e]:starts[e + 1]]
        per = -(-max(len(rows_e), 1) // CORES_PER_EXPERT)
        b1t = np.ascontiguousarray(b1[e].reshape(KH, 128).T)
        b2t = np.ascontiguousarray(b2[e].reshape(KH, 128).T)
        b3t = np.ascontiguousarray(b3[e].reshape(MO, 128).T)
        w1c = np.ascontiguousarray(W1[e])
        w2c = np.ascontiguousarray(W2[e])
        w3c = np.ascontiguousarray(W3[e])
        for h in range(CORES_PER_EXPERT):
            rows = rows_e[h * per:(h + 1) * per]
            xTc = np.zeros((D_IN, C), dtype=np.float32)
            if len(rows):
                xTc[:, :len(rows)] = xT[:, rows]
            in_maps.append({
                "xT": xTc,
                "w1": w1c, "w2": w2c, "w3": w3c,
                "b1t": b1t, "b2t": b2t, "b3t": b3t,
            })
            core_rows.append(rows)

    return nc, in_maps, core_rows, nrows


def _gather(results, core_rows, nrows):
    out = np.empty((nrows, D_OUT), dtype=np.float32)
    for c in range(N_CORES):
        rows = core_rows[c]
        if len(rows):
            out[rows] = results[c]["yT"][:, :len(rows)].T
    return out


def kernel(x, command, W1, b1, W2, b2, W3, b3):
    from concourse.bass_utils import run_bass_kernel_spmd

    nc, in_maps, core_rows, nrows = _prepare(
        x, command, W1, b1, W2, b2, W3, b3)
    res = run_bass_kernel_spmd(nc, in_maps, list(range(N_CORES)))
    return _gather(res.results, core_rows, nrows)


# revision 4
# speedup vs baseline: 1.1124x; 1.1124x over previous
"""MoE branch-routing kernel for Trainium2 (8 NeuronCores).

Strategy: expert-parallel with host-side routing. The batch is grouped by
`command` (4 experts); each expert's rows are split across 2 of the 8 cores.
Every core runs the same static SPMD program: a dense 3-layer MLP
(relu(xW1+b1) -> relu(hW2+b2) -> tanh(hW3+b3)) over C rows with ITS OWN
expert's weights delivered via its per-core input map. Activations stay in
[feature, row] layout on-chip so no transposes are needed; the host
transposes x in / y out and scatters rows back to their original positions.

Matmuls run as float32r (full 1 cycle/row PE rate for free dim >= 256,
~1e-4 relative precision vs fp32).
"""

import numpy as np

B, D_IN, H, D_OUT, E = 16384, 512, 1024, 512, 4
N_CORES = 8
CORES_PER_EXPERT = N_CORES // E
KI, KH, MO = D_IN // 128, H // 128, D_OUT // 128  # 4, 8, 4

_CACHE = {}


def _make_blocks(C):
    """Split C rows into matmul moving-dim blocks <=512, preferring >=256
    (float32r runs at full PE rate only when the moving dim is >=256)."""
    blocks, r = [], C
    while r > 0:
        if r >= 768 or r <= 512:
            nb = min(512, r)
        else:  # 512 < r < 768: split evenly so both pieces are >= 256
            nb = (r // 2 + 31) // 32 * 32
        blocks.append(nb)
        r -= nb
    return blocks


def _build_program(C):
    import concourse.tile as tile
    from concourse import bacc, mybir

    f32, f32r = mybir.dt.float32, mybir.dt.float32r
    AFT = mybir.ActivationFunctionType

    nc = bacc.Bacc("TRN2", target_bir_lowering=False, debug=False,
                   num_devices=N_CORES)
    xT_d = nc.dram_tensor("xT", [D_IN, C], f32, kind="ExternalInput").ap()
    w1_d = nc.dram_tensor("w1", [D_IN, H], f32, kind="ExternalInput").ap()
    w2_d = nc.dram_tensor("w2", [H, H], f32, kind="ExternalInput").ap()
    w3_d = nc.dram_tensor("w3", [H, D_OUT], f32, kind="ExternalInput").ap()
    b1_d = nc.dram_tensor("b1t", [128, KH], f32, kind="ExternalInput").ap()
    b2_d = nc.dram_tensor("b2t", [128, KH], f32, kind="ExternalInput").ap()
    b3_d = nc.dram_tensor("b3t", [128, MO], f32, kind="ExternalInput").ap()
    yT_d = nc.dram_tensor("yT", [D_OUT, C], f32, kind="ExternalOutput").ap()

    x_re = xT_d.rearrange("(k p) c -> p k c", p=128).bitcast(f32r)
    w1_re = w1_d.rearrange("(k p) h -> p k h", p=128).bitcast(f32r)
    w2_re = w2_d.rearrange("(k p) h -> p k h", p=128).bitcast(f32r)
    w3_re = w3_d.rearrange("(k p) h -> p k h", p=128).bitcast(f32r)
    y_re = yT_d.rearrange("(m p) c -> p m c", p=128)

    blocks = _make_blocks(C)

    block_off = []
    n0 = 0
    for nb in blocks:
        block_off.append(n0)
        n0 += nb

    with tile.TileContext(nc) as tc:
        with tc.tile_pool(name="wpool", bufs=1) as wpool, \
             tc.tile_pool(name="xpool", bufs=3) as xpool, \
             tc.tile_pool(name="hpool", bufs=2) as hpool, \
             tc.tile_pool(name="ypool", bufs=2) as ypool, \
             tc.tile_pool(name="psum", bufs=8, space="PSUM") as psum:

            xtiles = {}

            def load_x(b):
                if b >= len(blocks):
                    return
                xt = xpool.tile([128, KI, 512], f32r, name="x")
                nb, o = blocks[b], block_off[b]
                nc.sync.dma_start(xt[:, :, :nb], x_re[:, :, o:o + nb])
                xtiles[b] = xt

            # DMA emission order is queue order on the sync engine: get the
            # first x block in right after W1 so matmuls start at ~3us, and
            # interleave later x blocks between the big weight tensors.
            w1sb = wpool.tile([128, KI, H], f32r)
            for k in range(KI):
                nc.sync.dma_start(w1sb[:, k, :], w1_re[:, k, :])
            load_x(0)
            b1sb = wpool.tile([128, KH], f32)
            nc.sync.dma_start(b1sb[:], b1_d[:])
            b2sb = wpool.tile([128, KH], f32)
            nc.sync.dma_start(b2sb[:], b2_d[:])
            b3sb = wpool.tile([128, MO], f32)
            nc.sync.dma_start(b3sb[:], b3_d[:])
            w2sb = wpool.tile([128, KH, H], f32r)
            for k in range(KH):
                nc.sync.dma_start(w2sb[:, k, :], w2_re[:, k, :])
            load_x(1)
            w3sb = wpool.tile([128, KH, D_OUT], f32r)
            for k in range(KH):
                nc.sync.dma_start(w3sb[:, k, :], w3_re[:, k, :])
            load_x(2)

            for b, nb in enumerate(blocks):
                n0 = block_off[b]
                load_x(b + 3)
                xt = xtiles.pop(b)

                # L1: h1 = relu(x @ W1 + b1), laid out [H, rows]
                h1 = []
                for m in range(KH):
                    pt = psum.tile([128, 512], f32, name="ps")
                    for k in range(KI):
                        nc.tensor.matmul(
                            pt[:, :nb],
                            lhsT=w1sb[:, k, m * 128:(m + 1) * 128],
                            rhs=xt[:, k, :nb],
                            start=(k == 0), stop=(k == KI - 1))
                    ht = hpool.tile([128, 512], f32r, name=f"h1_{m}")
                    nc.scalar.activation(ht[:, :nb], pt[:, :nb], AFT.Relu,
                                         bias=b1sb[:, m:m + 1])
                    h1.append(ht)

                # L2: h2 = relu(h1 @ W2 + b2)
                h2 = []
                for m in range(KH):
                    pt = psum.tile([128, 512], f32, name="ps")
                    for k in range(KH):
                        nc.tensor.matmul(
                            pt[:, :nb],
                            lhsT=w2sb[:, k, m * 128:(m + 1) * 128],
                            rhs=h1[k][:, :nb],
                            start=(k == 0), stop=(k == KH - 1))
                    ht = hpool.tile([128, 512], f32r, name=f"h2_{m}")
                    nc.scalar.activation(ht[:, :nb], pt[:, :nb], AFT.Relu,
                                         bias=b2sb[:, m:m + 1])
                    h2.append(ht)

                # L3: y = tanh(h2 @ W3 + b3), DMA out per m-chunk
                for m in range(MO):
                    pt = psum.tile([128, 512], f32, name="ps")
                    for k in range(KH):
                        nc.tensor.matmul(
                            pt[:, :nb],
                            lhsT=w3sb[:, k, m * 128:(m + 1) * 128],
                            rhs=h2[k][:, :nb],
                            start=(k == 0), stop=(k == KH - 1))
                    yt = ypool.tile([128, 512], f32, name=f"y{m}")
                    nc.scalar.activation(yt[:, :nb], pt[:, :nb], AFT.Tanh,
                                         bias=b3sb[:, m:m + 1])
                    nc.sync.dma_start(y_re[:, m, n0:n0 + nb], yt[:, :nb])

    nc.compile()
    return nc


def _prepare(x, command, W1, b1, W2, b2, W3, b3):
    """Route rows to cores and build the per-core input maps.

    Returns (nc, in_maps, core_rows, nrows)."""
    x = np.ascontiguousarray(np.asarray(x, dtype=np.float32))
    command = np.asarray(command).astype(np.int64)
    W1 = np.asarray(W1, dtype=np.float32)
    b1 = np.asarray(b1, dtype=np.float32)
    W2 = np.asarray(W2, dtype=np.float32)
    b2 = np.asarray(b2, dtype=np.float32)
    W3 = np.asarray(W3, dtype=np.float32)
    b3 = np.asarray(b3, dtype=np.float32)

    nrows = x.shape[0]
    order = np.argsort(command, kind="stable")
    counts = np.bincount(command, minlength=E)
    starts = np.concatenate([[0], np.cumsum(counts)])

    # Static per-core row capacity, shared by all cores (one SPMD program).
    C = int(-(-int(counts.max()) // CORES_PER_EXPERT))
    C = max(256, -(-C // 64) * 64)

    if C not in _CACHE:
        _CACHE[C] = _build_program(C)
    nc = _CACHE[C]

    xT = x.T  # [D_IN, B] view
    in_maps = []
    core_rows = []
    for e in range(E):
        rows_e = order[starts[e]:starts[e + 1]]
        per = -(-max(len(rows_e), 1) // CORES_PER_EXPERT)
        b1t = np.ascontiguousarray(b1[e].reshape(KH, 128).T)
        b2t = np.ascontiguousarray(b2[e].reshape(KH, 128).T)
        b3t = np.ascontiguousarray(b3[e].reshape(MO, 128).T)
        w1c = np.ascontiguousarray(W1[e])
        w2c = np.ascontiguousarray(W2[e])
        w3c = np.ascontiguousarray(W3[e])
        for h in range(CORES_PER_EXPERT):
            rows = rows_e[h * per:(h + 1) * per]
            xTc = np.zeros((D_IN, C), dtype=np.float32)
            if len(rows):
                xTc[:, :len(rows)] = xT[:, rows]
            in_maps.append({
                "xT": xTc,
                "w1": w1c, "w2": w2c, "w3": w3c,
                "b1t": b1t, "b2t": b2t, "b3t": b3t,
            })
            core_rows.append(rows)

    return nc, in_maps, core_rows, nrows


def _gather(results, core_rows, nrows):
    out = np.empty((nrows, D_OUT), dtype=np.float32)
    for c in range(N_CORES):
        rows = core_rows[c]
        if len(rows):
            out[rows] = results[c]["yT"][:, :len(rows)].T
    return out


def kernel(x, command, W1, b1, W2, b2, W3, b3):
    from concourse.bass_utils import run_bass_kernel_spmd

    nc, in_maps, core_rows, nrows = _prepare(
        x, command, W1, b1, W2, b2, W3, b3)
    res = run_bass_kernel_spmd(nc, in_maps, list(range(N_CORES)))
    return _gather(res.results, core_rows, nrows)
